# revision 1
# baseline (speedup 1.0000x reference)
"""Trainium2 Bass kernel for DifferentiableNewmarkBeta.

Math: the Newmark-beta step is a linear recurrence on the 48-dim state
x=(u,v,a):  x_t = A x_{t-1} + Bm f_t, with A (48x48), Bm (48x16) built on the
host from M, C, K.  Outputs are the full state at every step.

Device algorithm (per core, batch-sharded 8 ways -> 8 batch rows x 8192 steps):
  - time split into 1024 chunks of L=8; chunk-local input convolutions are
    single matmuls contracting over 128 = (8 steps x 16 dof) partitions
  - hierarchical scan over chunk boundary states: 1024 -> 128 -> 16 -> serial
    (fp32 spine), then unwound back down
  - phase C expands every chunk to its 8 full states in one fused pair of
    fp16 matmuls per output species, producing output tiles already laid out
    as (128 rows=(batch,chunk), 128 free=(step,dof)) so every store is one
    contiguous 64KB DMA
Precision: scan spine fp32; bulk convolutions fp16 inputs with fp32 PSUM
accumulation (one-shot ~2.4e-4 input rounding, does not compound).
"""
import numpy as np

import concourse.bass as bass
import concourse.mybir as mybir
from concourse.bass_utils import run_bass_kernel_spmd
from concourse.tile import TileContext
from concourse.tile_rust import add_dep_helper

F32 = mybir.dt.float32
F16 = mybir.dt.float16

B, S, D = 64, 8192, 16
NCORE = 8
BL = B // NCORE          # 8 batch rows per core
L1, N1 = 8, 1024         # chunks
L2, N2 = 8, 128          # chunk groups
L3, N3 = 8, 16           # super groups
COLS = BL * N1           # 8192 matmul columns, col = b*N1 + c
DT_, BETA, GAMMA = 0.01, 0.25, 0.5


def _build_system(M, C, K):
    """A, Bm, Minv in float64."""
    M = M.astype(np.float64); C = C.astype(np.float64); K = K.astype(np.float64)
    dt, beta, gamma = DT_, BETA, GAMMA
    Minv = np.linalg.inv(M)
    Keffinv = np.linalg.inv(M + gamma * dt * C + beta * dt * dt * K)
    I = np.eye(16)
    Tup = np.zeros((16, 48)); Tvp = np.zeros((16, 48))
    Tup[:, 0:16] = I; Tup[:, 16:32] = dt * I; Tup[:, 32:48] = (0.5 - beta) * dt * dt * I
    Tvp[:, 16:32] = I; Tvp[:, 32:48] = (1.0 - gamma) * dt * I
    Ta = -Keffinv @ (C @ Tvp + K @ Tup)
    A = np.zeros((48, 48)); Bm = np.zeros((48, 16))
    A[32:48] = Ta
    A[0:16] = Tup + beta * dt * dt * Ta
    A[16:32] = Tvp + gamma * dt * Ta
    Bm[32:48] = Keffinv
    Bm[0:16] = beta * dt * dt * Keffinv
    Bm[16:32] = gamma * dt * Keffinv
    return Minv, A, Bm


def _build_weights(A, Bm):
    """Pack all device weight tensors. Returns (w16, w32)."""
    Apow = [np.eye(48)]
    for _ in range(L1):
        Apow.append(A @ Apow[-1])
    P = Apow[L1]
    P2 = np.linalg.matrix_power(P, L2)
    P3 = np.linalg.matrix_power(P2, L3)

    w16 = np.zeros((128, 1536), np.float64)
    # [0:48] Wfin : chunk-final conv, [(k,d),m] = (A^{7-k} Bm)[m,d]
    for k in range(L1):
        w16[k * 16:(k + 1) * 16, 0:48] = (Apow[L1 - 1 - k] @ Bm).T
    # [48:432] Wg per species, [432:816] Ws per species (rows 0:48)
    for sp in range(3):
        gcol = 48 + 128 * sp
        scol = 432 + 128 * sp
        for i in range(L1):
            blk = Apow[i + 1][sp * 16:(sp + 1) * 16, :]          # (16,48)
            w16[0:48, scol + i * 16:scol + (i + 1) * 16] = blk.T
            for k in range(i + 1):
                gblk = (Apow[i - k] @ Bm)[sp * 16:(sp + 1) * 16, :]
                w16[k * 16:(k + 1) * 16, gcol + i * 16:gcol + (i + 1) * 16] = gblk.T

    for k in range(L2):
        w16[0:48, 816 + 48 * k:816 + 48 * k + 48] = np.linalg.matrix_power(P, L2 - 1 - k).T
    w16[0:48, 1200:1248] = P.T
    I48 = np.eye(48)
    w16[0:48, 1248:1296] = P.T; w16[0:48, 1296:1344] = (P @ P).T
    w16[0:48, 1344:1392] = I48; w16[0:48, 1392:1440] = P.T
    w16[0:48, 1488:1536] = I48
    w32 = np.zeros((48, 1592), np.float64)
    for k in range(L2):
        w32[:, 48 * k:48 * k + 48] = np.linalg.matrix_power(P, L2 - 1 - k).T
    for k in range(L3):
        w32[:, 384 + 48 * k:384 + 48 * k + 48] = np.linalg.matrix_power(P2, L3 - 1 - k).T
    w32[:, 768:816] = P3.T
    w32[:, 816:864] = P2.T
    w32[:, 864:912] = P.T
    I48 = np.eye(48)
    w32[:, 920:968] = P3.T;  w32[:, 968:1016] = (P3 @ P3).T
    w32[:, 1016:1064] = I48; w32[:, 1064:1112] = P3.T
    w32[:, 1160:1208] = I48
    w32[:, 1208:1256] = P2.T; w32[:, 1256:1304] = (P2 @ P2).T
    w32[:, 1304:1352] = I48;  w32[:, 1352:1400] = P2.T
    w32[:, 1448:1496] = I48
    w32[:, 1496:1544] = P.T;  w32[:, 1544:1592] = (P @ P).T
    return w16.astype(np.float16), w32.astype(np.float32)


def _build_program():
    nc = bass.Bass(num_swdge_queues=4)
    g16 = nc.declare_dram_parameter("g16", [128, 1536 + COLS], F16, isOutput=False)
    w32x = nc.declare_dram_parameter("w32x", [48, 1600], F32, isOutput=False)
    _ocols = (2048, 3072, 3072)
    zout = [[nc.declare_dram_parameter(f"z{sp}_{g}", [128, _ocols[g]], F16, isOutput=True)
             for g in range(3)] for sp in range(3)]

    with TileContext(nc) as tc:
        with tc.tile_pool(name="const", bufs=1) as cpool, \
             tc.tile_pool(name="ps", bufs=2, space="PSUM") as pbig, \
             tc.tile_pool(name="psc", bufs=6, space="PSUM") as pc:
            GsD = cpool.tile([128, 1536 + COLS], F16, tag="bigC")  # DMA-landed
            WD32 = cpool.tile([48, 1600], F32)
            W16 = cpool.tile([128, 1536], F16)
            W32 = cpool.tile([48, 1592], F32)
            X0 = cpool.tile([48, BL], F32)
            SCR = cpool.tile([1, 8], F32)         # absorber scratch
            T1h = cpool.tile([48, COLS], F16)     # chunk-local finals (fp16)
            T2 = cpool.tile([48, BL * N2], F32)
            T3 = cpool.tile([48, BL * N3], F32)
            S3 = cpool.tile([48, BL * N3], F32)   # super-group start states
            S2 = cpool.tile([48, BL * N2], F32)
            S16 = cpool.tile([48, COLS], F16)     # chunk start states (fp16)

            _dmas = []
            _half = 1536 + 4096
            _dmas.append(nc.sync.dma_start(out=GsD[:, 0:_half], in_=g16[:, 0:_half]))
            _dmas.append(nc.sync.dma_start(out=GsD[:, _half:], in_=g16[:, _half:]))
            _dmas.append(nc.sync.dma_start(out=WD32[:], in_=w32x[:]))
            nc.vector.tensor_copy(out=W16[:], in_=GsD[:, 0:1536])
            nc.vector.tensor_copy(out=W32[:], in_=WD32[:, 0:1592])
            nc.vector.tensor_copy(out=X0[:], in_=WD32[:, 1592:1600])

            t1v = T1h[:].rearrange("p (b c k) -> p b c k", b=BL, c=N2, k=L2)
            t2v = T2[:].rearrange("p (b c k) -> p b c k", b=BL, c=N3, k=L3)
            s3v = S3[:].rearrange("p (b c) -> p b c", b=BL, c=N3)
            t3v = T3[:].rearrange("p (b c) -> p b c", b=BL, c=N3)
            s2v = S2[:].rearrange("p (b c k) -> p b c k", b=BL, c=N3, k=L3)
            s1v = S16[:].rearrange("p (b c k) -> p b c k", b=BL, c=N2, k=L2)
            s2f = S2[:].rearrange("p (b c) -> p b c", b=BL, c=N2)

            def gcol(o):
                return GsD[:, 1536 + o:1536 + o + 512]

            def front(h):
                """A1 + t2 for one batch half."""
                hb = slice(4 * h, 4 * h + 4)
                co = 4096 * h
                # dummy matmul: PE observes this half's G-DMA lane
                psd = pbig.tile([1, 1], F32, tag="ps")
                nc.tensor.matmul(out=psd[:], lhsT=GsD[0:1, 1536 + co:1537 + co],
                                 rhs=GsD[0:1, 1536 + co:1537 + co], start=True, stop=True)
                for n in range(8):
                    o = co + 512 * n
                    ps = pbig.tile([48, 512], F32, tag="ps")
                    nc.tensor.matmul(out=ps[:], lhsT=W16[:, 0:48],
                                     rhs=gcol(o), start=True, stop=True)
                    nc.vector.tensor_copy(out=T1h[:, o:o + 512], in_=ps[:])
                ps = pbig.tile([48, 512], F32, tag="ps")
                for k in range(L2):
                    nc.tensor.matmul(out=ps[:], lhsT=W16[0:48, 816 + 48 * k:816 + 48 * k + 48],
                                     rhs=t1v[:, hb, :, k], start=(k == 0), stop=(k == L2 - 1))
                nc.vector.tensor_copy(out=T2[:, 512 * h:512 * h + 512], in_=ps[:])

            def midscan():
                """t3 conv + top scan + unwind3, both halves together."""
                ps3 = pbig.tile([48, 128], F32, tag="ps")
                for k in range(L3):
                    nc.tensor.matmul(out=ps3[:], lhsT=W32[:, 384 + 48 * k:384 + 48 * k + 48],
                                     rhs=t2v[:, :, :, k], start=(k == 0), stop=(k == L3 - 1))
                nc.vector.tensor_copy(out=T3[:], in_=ps3[:])
                nc.vector.tensor_copy(out=SCR[0:1, 0:1], in_=T3[0:1, 0:1])
                for c3 in range(1, N3):
                    ps = pbig.tile([48, BL], F32, tag="ps")
                    rhs = X0[:] if c3 == 1 else s3v[:, :, c3 - 1]
                    nc.tensor.matmul(out=ps[:], lhsT=W32[:, 768:816],
                                     rhs=rhs, start=True, stop=True)
                    nc.vector.tensor_add(out=s3v[:, :, c3], in0=ps[:], in1=t3v[:, :, c3 - 1])
                nc.vector.tensor_copy(out=s3v[:, :, 0], in_=X0[:])
                for k in range(L3 - 1):
                    ps = pbig.tile([48, 128], F32, tag="ps")
                    rhs = S3[:] if k == 0 else s2v[:, :, :, k]
                    nc.tensor.matmul(out=ps[:], lhsT=W32[:, 816:864],
                                     rhs=rhs, start=True, stop=True)
                    nc.vector.tensor_add(out=s2v[:, :, :, k + 1], in0=ps[:], in1=t2v[:, :, :, k])
                nc.vector.tensor_copy(out=s2v[:, :, :, 0], in_=s3v[:, :, :])

            def unwind2(h):
                hb = slice(4 * h, 4 * h + 4)
                for k in range(L2 - 1):
                    ps = pbig.tile([48, 512], F32, tag="ps")
                    if k == 0:
                        nc.tensor.matmul(out=ps[:], lhsT=W32[:, 864:912],
                                         rhs=s2f[:, hb, :], start=True, stop=True)
                    else:
                        nc.tensor.matmul(out=ps[:], lhsT=W16[0:48, 1200:1248],
                                         rhs=s1v[:, hb, :, k], start=True, stop=True)
                    nc.vector.tensor_add(out=s1v[:, hb, :, k + 1], in0=ps[:],
                                         in1=t1v[:, hb, :, k])
                nc.vector.tensor_copy(out=s1v[:, hb, :, 0], in_=s2f[:, hb, :])

            OSTI = cpool.tile([128, 3 * COLS], F16)
            ostv = OSTI[:].rearrange("p (s t e) -> p s t e", s=3, t=64, e=128)
            _lasts = []

            def phase_c(h):
                for t in range(32 * h, 32 * h + 32):
                    cs = slice(128 * t, 128 * t + 128)
                    ps = pc.tile([128, 384], F32, tag="psc")
                    nc.tensor.matmul(out=ps[:], lhsT=S16[:, cs],
                                     rhs=W16[0:48, 432:816], start=True, stop=False)
                    mm = nc.tensor.matmul(out=ps[:], lhsT=GsD[:, 1536 + 128 * t:1536 + 128 * (t + 1)],
                                          rhs=W16[:, 48:432], start=False, stop=True)
                    if t == 32 * h:
                        nc.vector.tensor_copy(out=SCR[0:1, 1:2], in_=ps[0:1, 0:1])
                    cp = nc.vector.tensor_copy(out=ostv[:, :, t, :], in_=ps[:])
                    if t == 32 * h + 31:
                        _lasts.extend([mm, cp])
                    _flush = {15: (0, 0, 16), 39: (1, 16, 40), 63: (2, 40, 64)}
                    if t in _flush:
                        fi, t0, t1 = _flush[t]
                        for sp in range(3):
                            eng = nc.scalar if (fi * 3 + sp) < 5 else nc.gpsimd
                            _dmas.append(eng.dma_start(
                                out=zout[sp][fi][:],
                                in_=OSTI[:, COLS * sp + 128 * t0:COLS * sp + 128 * t1]))

            front(0)
            front(1)
            midscan()
            unwind2(0)
            unwind2(1)
            phase_c(0)
            phase_c(1)

            # sequencer observes each DMA lane + PE/DVE tails so the exit
            # drain's wait list collapses under the walrus 1-wait limit
            for _d in _dmas + _lasts:
                _n = nc.sync.nop()
                add_dep_helper(_n.ins, _d.ins, sync=True, reason="drain fan-in")
    return nc


_PROG = None


def kernel(F, M, C, K):
    global _PROG
    F = np.ascontiguousarray(np.asarray(F, np.float32))
    Minv, A, Bm = _build_system(np.asarray(M), np.asarray(C), np.asarray(K))
    w16, w32 = _build_weights(A, Bm)

    a0 = (F[:, 0, :].astype(np.float64) @ Minv.T).astype(np.float32)
    G = np.concatenate([F[:, 1:, :], np.zeros((B, 1, D), np.float32)], axis=1)

    if _PROG is None:
        _PROG = _build_program()
    nc = _PROG

    in_maps = []
    for c in range(NCORE):
        sl = slice(c * BL, (c + 1) * BL)
        g16 = np.ascontiguousarray(
            G[sl].reshape(BL, N1, 128).reshape(COLS, 128).astype(np.float16).T)
        x0 = np.zeros((48, BL), np.float32)
        x0[32:48, :] = a0[sl].T
        w32x = np.zeros((48, 1600), np.float32)
        w32x[:, 0:1592] = w32
        w32x[:, 1592:1600] = x0
        gext = np.concatenate([w16, g16], axis=1)
        in_maps.append({"g16": np.ascontiguousarray(gext), "w32x": w32x})

    res = run_bass_kernel_spmd(nc, in_maps, list(range(NCORE))).results

    u = np.zeros((B, S, D), np.float32)
    v = np.zeros((B, S, D), np.float32)
    a = np.zeros((B, S, D), np.float32)
    for c in range(NCORE):
        sl = slice(c * BL, (c + 1) * BL)
        for sp, arr in enumerate((u, v, a)):
            full = np.concatenate([res[c][f"z{sp}_{g}"] for g in range(3)],
                                  axis=1).astype(np.float32)
            zz = full.reshape(128, 64, 128).transpose(1, 0, 2).reshape(COLS, 128)
            z = zz.reshape(BL, N1, L1, 16).reshape(BL, S, 16)
            arr[sl, 1:, :] = z[:, :S - 1, :]
    a[:, 0, :] = a0
    return (u, v, a)

